# revision 33
# baseline (speedup 1.0000x reference)
"""Trainium2 Bass kernel for nn_Efficient8BitALU_AddSub.

Contract: kernel(**inputs) takes FULL unsharded inputs (numpy), returns FULL
output [32, 2048, 128] float32.  Internally shards tokens across 8 NeuronCores
(pure data parallel), runs a Bass/Tile kernel per core, gathers.

v2 design (per core, 8192 tokens = 64 tiles of 128):
  DMA   p-major layout: token = p*nt + n, so every chunk is 4KB-contiguous
        per partition in HBM.
  Pool  decode compare (tsel) + window min-reduce; final one-hot + scatter.
  DVE   decode fixup, flags, c-vector assembly (bf16, padded to 128 comps:
        pos0 comps at cols 0..13, pos1 at cols 32..45, col 4 = 1.0).
  XBAR  one dma_start_transpose per 16-tile batch turns c token-major
        [128, B*128] into comp-major tiles [128, B, 128] — no PE transposes,
        no psum drain copies.
  PE    h = W16^T c per (tile,pos): K=14 bf16 hi/lo-split weights (exact to
        ~1e-5), N=128; relu via ACT (psum->SBUF fp16); layer2 LDW(RH)+
        matmul(W2, N=2) -> res[tok, (tile,pos,add/sub)] token-major psum.
  DVE   select by is_add, RNE round, clamp, fold processed mask.
  Pool  one-hot is_equal vs iota, fused scatter-add into x.
"""

import sys

import numpy as np

sys.path.insert(0, "/opt/trn_rl_repo")

import ml_dtypes  # noqa: E402
import concourse.bacc as bacc  # noqa: E402
import concourse.bass as bass  # noqa: E402
import concourse.mybir as mybir  # noqa: E402
import concourse.tile as tile  # noqa: E402

dt = mybir.dt
Alu = mybir.AluOpType
Act = mybir.ActivationFunctionType

# ---- problem constants (hardcoded per contract) ----
B, S, D = 32, 2048, 128
NCORES = 8
TOK = B * S                   # 65536
TPC = TOK // NCORES           # 8192 tokens per core

MARK_AX, OP_ADD, OP_SUB = 0, 1, 2
WIN0 = 3                      # 4 contiguous 16-wide decode windows: 3..66
OUT_LO = 67                   # outputs 67..98 (lo 67:83, hi 83:99)
OPA, OPS = 124, 125
GE_RESULT = 63
ROUND_C = 12582912.0          # 1.5 * 2**23 : RNE round-to-integer magic

BT = 16                       # tiles per batch (DVE/Pool/XBAR granularity)
BK = 4                        # tiles per PE block (psum granularity)


def build_nc(tpc=TPC, bt=BT, bk=BK):
    nt = tpc // 128
    nbatch = nt // bt
    kpb = bt // bk            # PE blocks per batch
    assert nt % bt == 0 and bt % bk == 0

    nc = bacc.Bacc("TRN2", target_bir_lowering=False, debug=False,
                   num_devices=NCORES)
    xd = nc.dram_tensor("xc", [tpc, D], dt.float32, kind="ExternalInput")
    w16d = nc.dram_tensor("cW16", [128, 128], dt.bfloat16, kind="ExternalInput")
    w2d = nc.dram_tensor("cW2", [128, 2], dt.float16, kind="ExternalInput")
    iotad = nc.dram_tensor("cIOTA", [128, 32], dt.float16, kind="ExternalInput")
    k16d = nc.dram_tensor("cK16", [128, 64], dt.float16, kind="ExternalInput")
    yd = nc.dram_tensor("yc", [tpc, D], dt.float32, kind="ExternalOutput")

    # p-major: token = p * nt + n -> per-partition-contiguous DMA
    xr = xd.ap().rearrange("(p n) f -> p n f", p=128)
    yr = yd.ap().rearrange("(p n) f -> p n f", p=128)

    with tile.TileContext(nc) as tc:
        with (
            tc.tile_pool(name="const", bufs=1) as cpool,
            tc.tile_pool(name="xbuf", bufs=1) as xpool,
            tc.tile_pool(name="work", bufs=3) as wpool,
            tc.tile_pool(name="hp", bufs=2, space="PSUM") as hp_pool,
            tc.tile_pool(name="rp", bufs=2, space="PSUM") as rp_pool,
        ):
            W16 = cpool.tile([128, 128], dt.bfloat16, tag="w16")
            W2 = cpool.tile([128, 2], dt.float16, tag="w2")
            IOTA = cpool.tile([128, 32], dt.float16, tag="iota")

            X = xpool.tile([128, nt * 128], dt.float32, tag="X")
            XR = X[:].rearrange("p (n f) -> p n f", f=128)

            K16S = xpool.tile([128, bt * 64], dt.float16, tag="K16S")
            nc.gpsimd.dma_start(
                K16S[:].rearrange("p (n k) -> p n k", k=64),
                k16d.ap()[:, None, :].to_broadcast([128, bt, 64]))
            nc.gpsimd.dma_start(W16[:], w16d.ap())
            nc.gpsimd.dma_start(W2[:], w2d.ap())
            nc.gpsimd.dma_start(IOTA[:], iotad.ap())

            # c staging (bf16, 128 comp cols per tile: pos0 at 0..13, pos1 at
            # 32..45, col 4 of each = 1.0, rest zero) + comp-major mirror
            cbs, cts = [], []
            for i in range(2):
                cb = xpool.tile([128, bt * 128], dt.bfloat16, tag=f"CB{i}",
                                name=f"CB{i}")
                nc.vector.memset(cb[:], 0.0)
                cb4 = cb[:].rearrange("p (n q c) -> p n q c", q=4, c=32)
                nc.vector.memset(cb4[:, :, 0:2, 4:5], 1.0)
                cbs.append(cb)
                cts.append(xpool.tile([128, bt * 128], dt.bfloat16,
                                      tag=f"CT{i}", name=f"CT{i}"))

            rhs_ = [xpool.tile([128, bk * 256], dt.float16, tag=f"RH{i}",
                               name=f"RH{i}") for i in range(3)]

            # input: 8-tile chunks on the software DGE (Pool ring) so the
            # chained issues never block the sync/scalar HWDGE rings that
            # carry the XBAR transposes and output stores.  2 chains so
            # early chunks get full bandwidth and land first.
            chunk = 8
            prev_in = [None, None]
            for d_ in range(nt // chunk):
                t0 = d_ * chunk
                di = nc.gpsimd.dma_start(XR[:, t0:t0 + chunk, :],
                                         xr[:, t0:t0 + chunk, :])
                if prev_in[d_ % 2] is not None:
                    tile.add_dep_helper(di.ins, prev_in[d_ % 2].ins,
                                        reason="input chunk ordering")
                prev_in[d_ % 2] = di

            flgs = [None] * nbatch     # per-batch flag tiles for post phase
            rsel = [None] * nbatch
            rps = [None] * nbatch
            pend_l2 = []               # lagged layer2 blocks

            def decode(b):
                T0 = b * bt
                CB = cbs[b % 2]
                CB4 = CB[:].rearrange("p (n q c) -> p n q c", q=4, c=32)
                CT = cts[b % 2]

                # ---------- decode (DVE; gpsimd tensor ops are ucode-slow) --
                # tsel in fp16 so the min-reduce runs in the 2x DVE mode
                TSEL = wpool.tile([128, bt * 64], dt.float16, tag="tsel")
                nc.vector.scalar_tensor_tensor(
                    out=TSEL[:],
                    in0=XR[:, T0:T0 + bt, WIN0:WIN0 + 64],
                    scalar=0.5,
                    in1=K16S[:].rearrange("p (n k) -> p n k", k=64),
                    op0=Alu.is_gt, op1=Alu.mult)
                NIB = wpool.tile([128, bt * 4], dt.float16, tag="nib")
                nc.vector.tensor_reduce(
                    out=NIB[:],
                    in_=TSEL[:].rearrange("p (n w k) -> p n w k", w=4, k=16),
                    axis=mybir.AxisListType.X, op=Alu.min)
                NM = wpool.tile([128, bt * 4], dt.float16, tag="nm")
                nc.vector.tensor_scalar(out=NM[:], in0=NIB[:], scalar1=-0.5,
                                        scalar2=None, op0=Alu.is_lt)
                nc.vector.scalar_tensor_tensor(out=NIB[:], in0=NIB[:],
                                               scalar=16.0, in1=NM[:],
                                               op0=Alu.add, op1=Alu.mult)
                NIB4 = NIB[:].rearrange("p (n w) -> p n w", w=4)
                NIBV = NIB4.rearrange("p n (ab pos) -> p n pos ab", pos=2)

                # ---------- flags (c-major layout -> contiguous slices) ----
                FLG = wpool.tile([128, 3 * bt], dt.float32, tag="flg")
                FLG3 = FLG[:].rearrange("p (c n) -> p c n", c=3)
                nc.vector.tensor_scalar(
                    out=FLG3,
                    in0=XR[:, T0:T0 + bt, 0:3].rearrange("p n c -> p c n"),
                    scalar1=0.5, scalar2=None, op0=Alu.is_gt)
                MA = FLG3[:, 1, :]
                M2 = wpool.tile([128, bt], dt.float32, tag="m2")
                nc.vector.tensor_tensor(out=M2[:], in0=MA, in1=FLG3[:, 2, :],
                                        op=Alu.max)
                nc.vector.scalar_tensor_tensor(out=M2[:], in0=M2[:], scalar=2.0,
                                               in1=FLG3[:, 0, :], op0=Alu.mult,
                                               op1=Alu.mult)
                flgs[b] = (FLG, M2)

                OPV = XR[:, T0:T0 + bt, OPA:OPS + 1][:, :, None, :] \
                    .broadcast_to([128, bt, 2, 2])

                # ---------- c build (bf16) ----------
                # cols 0..4 = [a, b, opA, opS, 1]; one fused multiply makes
                # cols 5..9 = cols 0..4 * mA (col 9 = mA since col 4 == 1);
                # cols 10..13 duplicate [a, b, a*mA, b*mA] for the lo-split
                # weight rows.
                nc.vector.tensor_copy(CB4[:, :, 0:2, 0:2], NIBV)
                nc.vector.tensor_copy(CB4[:, :, 0:2, 2:4], OPV)
                nc.vector.tensor_tensor(
                    out=CB4[:, :, 0:2, 5:10], in0=CB4[:, :, 0:2, 0:5],
                    in1=MA[:, :, None, None].broadcast_to([128, bt, 2, 5]),
                    op=Alu.mult)
                nc.vector.tensor_copy(CB4[:, :, 0:2, 10:12], CB4[:, :, 0:2, 0:2])
                nc.vector.tensor_copy(CB4[:, :, 0:2, 12:14], CB4[:, :, 0:2, 5:7])

                # ---------- comp-major via XBAR (one instruction) ----------
                nc.sync.dma_start_transpose(
                    CT[:].rearrange("p (n f) -> p n f", f=128), CB[:])

                rps[b] = rp_pool.tile([128, bt * 4], dt.float32, tag="rp",
                                      name=f"rp{b}")

            def emit_l2(args):
                b, k, RH = args
                for pos in range(2):
                    for j in range(bk):
                        c0 = pos * (bk * 128) + j * 128
                        lc = (k * bk + j) * 4 + pos * 2
                        nc.tensor.matmul(
                            rps[b][:, lc:lc + 2],
                            RH[:, c0:c0 + 128],
                            W2[:],
                            start=True, stop=True)

            def block(b, k):
                CT3 = cts[b % 2][:].rearrange("p (n f) -> p n f", f=128)
                hp = hp_pool.tile([128, bk * 256], dt.float32, tag="hp")
                for pos in range(2):
                    r0 = 32 * pos
                    for j in range(bk):
                        nc.tensor.matmul(
                            hp[:, pos * (bk * 128) + j * 128:
                               pos * (bk * 128) + j * 128 + 128],
                            W16[r0:r0 + 14, :],
                            CT3[r0:r0 + 14, k * bk + j, :],
                            start=True, stop=True,
                            tile_position=(r0, 0))
                RH = rhs_[(b * kpb + k) % 3]
                nc.scalar.activation(RH[:], hp[:], Act.Relu)
                return (b, k, RH)

            def post(b):
                # rp cols per (tile,pos): [res_add - res_sub, res_sub]
                # (difference baked into W2 on the host)
                T0 = b * bt
                FLG, M2 = flgs[b]
                MA = FLG[:].rearrange("p (c n) -> p c n", c=3)[:, 1, :]
                RESS = wpool.tile([128, bt * 4], dt.float32, tag="ress")
                nc.vector.tensor_copy(RESS[:], rps[b][:])
                RESV = RESS[:].rearrange("p (n s w) -> p n s w", s=2, w=2)
                RSEL = wpool.tile([128, bt * 2], dt.float32, tag="rsel")
                RSV = RSEL[:].rearrange("p (n s) -> p n s", s=2)
                # rsel = diff*mA + res_sub
                nc.vector.tensor_tensor(
                    out=RSV, in0=RESV[:, :, :, 0],
                    in1=MA[:, :, None].broadcast_to([128, bt, 2]), op=Alu.mult)
                nc.vector.tensor_tensor(out=RSV, in0=RSV,
                                        in1=RESV[:, :, :, 1], op=Alu.add)
                nc.vector.tensor_scalar(out=RSEL[:], in0=RSEL[:],
                                        scalar1=ROUND_C,
                                        scalar2=ROUND_C - 100.0,
                                        op0=Alu.add, op1=Alu.subtract)
                nc.vector.tensor_scalar(out=RSEL[:], in0=RSEL[:],
                                        scalar1=100.0, scalar2=115.0,
                                        op0=Alu.max, op1=Alu.min)
                RS16 = wpool.tile([128, bt * 2], dt.float16, tag="rs16")
                RS16V = RS16[:].rearrange("p (n s) -> p n s", s=2)
                nc.vector.scalar_tensor_tensor(
                    out=RS16[:].rearrange("p (n s) -> p n s", s=2),
                    in0=M2[:, :, None].broadcast_to([128, bt, 2]),
                    scalar=-50.0, in1=RSV, op0=Alu.mult, op1=Alu.add)
                rsel[b] = RS16
                EQ = wpool.tile([128, bt * 32], dt.float16, tag="eq")
                nc.vector.tensor_tensor(
                    out=EQ[:],
                    in0=IOTA[:].rearrange("p (s k) -> p s k", s=2)[:, None]
                        .broadcast_to([128, bt, 2, 16]),
                    in1=RS16V[:, :, :, None].broadcast_to([128, bt, 2, 16]),
                    op=Alu.is_equal)
                nc.vector.scalar_tensor_tensor(
                    out=XR[:, T0:T0 + bt, OUT_LO:OUT_LO + 32],
                    in0=EQ[:].rearrange("p (n c) -> p n c", c=32),
                    scalar=2.0,
                    in1=XR[:, T0:T0 + bt, OUT_LO:OUT_LO + 32],
                    op0=Alu.mult, op1=Alu.add)
                nc.sync.dma_start(yr[:, T0:T0 + bt, :], XR[:, T0:T0 + bt, :])

            for b in range(nbatch):
                decode(b)
                for k in range(kpb):
                    blk = block(b, k)
                    if pend_l2:
                        emit_l2(pend_l2.pop(0))
                    pend_l2.append(blk)
                if b >= 1:
                    post(b - 1)
            while pend_l2:
                emit_l2(pend_l2.pop(0))
            post(nbatch - 1)

    nc.compile()
    return nc


def make_consts(W_add1, b_add1, W_add2, b_add2, W_sub1, b_sub1, W_sub2, b_sub2):
    f32 = np.float32
    bf16 = ml_dtypes.bfloat16
    rows = [0, 1, 27, 28]     # GE comps: NIB_A, NIB_B, OP_START+25, OP_START+26

    def eff(W1, b1):
        return np.concatenate([np.asarray(W1, f32)[rows, :],
                               np.asarray(b1, f32)[None, :]], axis=0)

    es = eff(W_sub1, b_sub1)
    ea = eff(W_add1, b_add1)
    blk = np.zeros((10, 128), f32)
    blk[0:5] = es
    blk[5:10] = (ea.astype(np.float64) - es.astype(np.float64)).astype(f32)
    bhi = blk.astype(bf16)
    blo = (blk - bhi.astype(f32)).astype(bf16)
    w16 = np.zeros((128, 128), bf16)
    for s in range(4):
        w16[32 * s:32 * s + 10] = bhi
        w16[32 * s + 10:32 * s + 14] = blo[[0, 1, 5, 6]]

    w2a = np.asarray(W_add2, np.float64)[:, GE_RESULT]
    w2s = np.asarray(W_sub2, np.float64)[:, GE_RESULT]
    w2 = np.stack([w2a - w2s, w2s], axis=1).astype(f32).astype(np.float16)

    iota = np.broadcast_to(np.tile(np.arange(16, dtype=np.float16), 2),
                           (128, 32)).copy()
    k16 = np.broadcast_to(((np.arange(64, dtype=f32) % 16) - 16.0)
                          .astype(np.float16), (128, 64)).copy()
    return dict(cW16=w16, cW2=w2, cIOTA=iota, cK16=k16)


_NC_CACHE = {}


def _get_nc(tpc=TPC, bt=BT, bk=BK):
    key = (tpc, bt, bk)
    if key not in _NC_CACHE:
        _NC_CACHE[key] = build_nc(tpc, bt, bk)
    return _NC_CACHE[key]


def kernel(x_bd, W_add1, b_add1, W_add2, b_add2, W_sub1, b_sub1, W_sub2, b_sub2):
    from concourse import bass_utils

    x = np.ascontiguousarray(np.asarray(x_bd, dtype=np.float32)).reshape(TOK, D)
    consts = make_consts(W_add1, b_add1, W_add2, b_add2,
                         W_sub1, b_sub1, W_sub2, b_sub2)
    badd2 = float(np.asarray(b_add2)[GE_RESULT])
    bsub2 = float(np.asarray(b_sub2)[GE_RESULT])
    assert badd2 == 0.0 and bsub2 == 0.0, "nonzero output bias not folded"

    nc = _get_nc()
    in_maps = []
    for c in range(NCORES):
        m = dict(consts)
        m["xc"] = x[c * TPC:(c + 1) * TPC]
        in_maps.append(m)
    res = bass_utils.run_bass_kernel_spmd(nc, in_maps, list(range(NCORES)))
    y = np.concatenate([res.results[c]["yc"] for c in range(NCORES)], axis=0)
    return y.reshape(B, S, D)


if __name__ == "__main__":
    build_nc()
    print("built ok")


# revision 34
# speedup vs baseline: 1.0217x; 1.0217x over previous
"""Trainium2 Bass kernel for nn_Efficient8BitALU_AddSub.

Contract: kernel(**inputs) takes FULL unsharded inputs (numpy), returns FULL
output [32, 2048, 128] float32.  Internally shards tokens across 8 NeuronCores
(pure data parallel), runs a Bass/Tile kernel per core, gathers.

v2 design (per core, 8192 tokens = 64 tiles of 128):
  DMA   p-major layout: token = p*nt + n, so every chunk is 4KB-contiguous
        per partition in HBM.
  Pool  decode compare (tsel) + window min-reduce; final one-hot + scatter.
  DVE   decode fixup, flags, c-vector assembly (bf16, padded to 128 comps:
        pos0 comps at cols 0..13, pos1 at cols 32..45, col 4 = 1.0).
  XBAR  one dma_start_transpose per 16-tile batch turns c token-major
        [128, B*128] into comp-major tiles [128, B, 128] — no PE transposes,
        no psum drain copies.
  PE    h = W16^T c per (tile,pos): K=14 bf16 hi/lo-split weights (exact to
        ~1e-5), N=128; relu via ACT (psum->SBUF fp16); layer2 LDW(RH)+
        matmul(W2, N=2) -> res[tok, (tile,pos,add/sub)] token-major psum.
  DVE   select by is_add, RNE round, clamp, fold processed mask.
  Pool  one-hot is_equal vs iota, fused scatter-add into x.
"""

import sys

import numpy as np

sys.path.insert(0, "/opt/trn_rl_repo")

import ml_dtypes  # noqa: E402
import concourse.bacc as bacc  # noqa: E402
import concourse.bass as bass  # noqa: E402
import concourse.mybir as mybir  # noqa: E402
import concourse.tile as tile  # noqa: E402

dt = mybir.dt
Alu = mybir.AluOpType
Act = mybir.ActivationFunctionType

# ---- problem constants (hardcoded per contract) ----
B, S, D = 32, 2048, 128
NCORES = 8
TOK = B * S                   # 65536
TPC = TOK // NCORES           # 8192 tokens per core

MARK_AX, OP_ADD, OP_SUB = 0, 1, 2
WIN0 = 3                      # 4 contiguous 16-wide decode windows: 3..66
OUT_LO = 67                   # outputs 67..98 (lo 67:83, hi 83:99)
OPA, OPS = 124, 125
GE_RESULT = 63
ROUND_C = 12582912.0          # 1.5 * 2**23 : RNE round-to-integer magic

BT = 16                       # tiles per batch (DVE/Pool/XBAR granularity)
BK = 4                        # tiles per PE block (psum granularity)


def build_nc(tpc=TPC, bt=BT, bk=BK):
    nt = tpc // 128
    nbatch = nt // bt
    kpb = bt // bk            # PE blocks per batch
    assert nt % bt == 0 and bt % bk == 0

    nc = bacc.Bacc("TRN2", target_bir_lowering=False, debug=False,
                   num_devices=NCORES)
    xd = nc.dram_tensor("xc", [tpc, D], dt.float32, kind="ExternalInput")
    w16d = nc.dram_tensor("cW16", [128, 128], dt.bfloat16, kind="ExternalInput")
    w2d = nc.dram_tensor("cW2", [128, 2], dt.float16, kind="ExternalInput")
    iotad = nc.dram_tensor("cIOTA", [128, 32], dt.float16, kind="ExternalInput")
    k16d = nc.dram_tensor("cK16", [128, 64], dt.float16, kind="ExternalInput")
    yd = nc.dram_tensor("yc", [tpc, D], dt.float32, kind="ExternalOutput")

    # p-major: token = p * nt + n -> per-partition-contiguous DMA
    xr = xd.ap().rearrange("(p n) f -> p n f", p=128)
    yr = yd.ap().rearrange("(p n) f -> p n f", p=128)

    with tile.TileContext(nc) as tc:
        with (
            tc.tile_pool(name="const", bufs=1) as cpool,
            tc.tile_pool(name="xbuf", bufs=1) as xpool,
            tc.tile_pool(name="work", bufs=3) as wpool,
            tc.tile_pool(name="hp", bufs=2, space="PSUM") as hp_pool,
            tc.tile_pool(name="rp", bufs=2, space="PSUM") as rp_pool,
        ):
            W16 = cpool.tile([128, 128], dt.bfloat16, tag="w16")
            W2 = cpool.tile([128, 2], dt.float16, tag="w2")
            IOTA = cpool.tile([128, 32], dt.float16, tag="iota")

            X = xpool.tile([128, nt * 128], dt.float32, tag="X")
            XR = X[:].rearrange("p (n f) -> p n f", f=128)

            K16S = xpool.tile([128, bt * 64], dt.float16, tag="K16S")
            nc.gpsimd.dma_start(
                K16S[:].rearrange("p (n k) -> p n k", k=64),
                k16d.ap()[:, None, :].to_broadcast([128, bt, 64]))
            nc.gpsimd.dma_start(W16[:], w16d.ap())
            nc.gpsimd.dma_start(W2[:], w2d.ap())
            nc.gpsimd.dma_start(IOTA[:], iotad.ap())

            # c staging (bf16, 128 comp cols per tile: pos0 at 0..13, pos1 at
            # 32..45, col 4 of each = 1.0, rest zero) + comp-major mirror
            cbs, cts = [], []
            for i in range(2):
                cb = xpool.tile([128, bt * 128], dt.bfloat16, tag=f"CB{i}",
                                name=f"CB{i}")
                nc.vector.memset(cb[:], 0.0)
                cb4 = cb[:].rearrange("p (n q c) -> p n q c", q=4, c=32)
                nc.vector.memset(cb4[:, :, 0:2, 4:5], 1.0)
                cbs.append(cb)
                cts.append(xpool.tile([128, bt * 128], dt.bfloat16,
                                      tag=f"CT{i}", name=f"CT{i}"))

            rhs_ = [xpool.tile([128, bk * 256], dt.float16, tag=f"RH{i}",
                               name=f"RH{i}") for i in range(3)]

            # input: 8-tile chunks on the scalar HWDGE ring (2 FIFO chains so
            # early chunks get full bandwidth and land first).  The chained
            # issues block only this ring; the sync ring stays free for the
            # XBAR transposes and output stores, and the ACT engine does no
            # compute until well after the issues drain.
            chunk = 8
            prev_in = [None, None]
            for d_ in range(nt // chunk):
                t0 = d_ * chunk
                di = nc.scalar.dma_start(XR[:, t0:t0 + chunk, :],
                                         xr[:, t0:t0 + chunk, :])
                if prev_in[d_ % 2] is not None:
                    tile.add_dep_helper(di.ins, prev_in[d_ % 2].ins,
                                        reason="input chunk ordering")
                prev_in[d_ % 2] = di

            flgs = [None] * nbatch     # per-batch flag tiles for post phase
            rsel = [None] * nbatch
            rps = [None] * nbatch
            pend_l2 = []               # lagged layer2 blocks

            def decode(b):
                T0 = b * bt
                CB = cbs[b % 2]
                CB4 = CB[:].rearrange("p (n q c) -> p n q c", q=4, c=32)
                CT = cts[b % 2]

                # ---------- decode (DVE; gpsimd tensor ops are ucode-slow) --
                # tsel in fp16 so the min-reduce runs in the 2x DVE mode
                TSEL = wpool.tile([128, bt * 64], dt.float16, tag="tsel")
                nc.vector.scalar_tensor_tensor(
                    out=TSEL[:],
                    in0=XR[:, T0:T0 + bt, WIN0:WIN0 + 64],
                    scalar=0.5,
                    in1=K16S[:].rearrange("p (n k) -> p n k", k=64),
                    op0=Alu.is_gt, op1=Alu.mult)
                NIB = wpool.tile([128, bt * 4], dt.float16, tag="nib")
                nc.vector.tensor_reduce(
                    out=NIB[:],
                    in_=TSEL[:].rearrange("p (n w k) -> p n w k", w=4, k=16),
                    axis=mybir.AxisListType.X, op=Alu.min)
                NM = wpool.tile([128, bt * 4], dt.float16, tag="nm")
                nc.vector.tensor_scalar(out=NM[:], in0=NIB[:], scalar1=-0.5,
                                        scalar2=None, op0=Alu.is_lt)
                nc.vector.scalar_tensor_tensor(out=NIB[:], in0=NIB[:],
                                               scalar=16.0, in1=NM[:],
                                               op0=Alu.add, op1=Alu.mult)
                NIB4 = NIB[:].rearrange("p (n w) -> p n w", w=4)
                NIBV = NIB4.rearrange("p n (ab pos) -> p n pos ab", pos=2)

                # ---------- flags (c-major layout -> contiguous slices) ----
                FLG = wpool.tile([128, 3 * bt], dt.float32, tag="flg")
                FLG3 = FLG[:].rearrange("p (c n) -> p c n", c=3)
                nc.vector.tensor_scalar(
                    out=FLG3,
                    in0=XR[:, T0:T0 + bt, 0:3].rearrange("p n c -> p c n"),
                    scalar1=0.5, scalar2=None, op0=Alu.is_gt)
                MA = FLG3[:, 1, :]
                M2 = wpool.tile([128, bt], dt.float32, tag="m2")
                nc.vector.tensor_tensor(out=M2[:], in0=MA, in1=FLG3[:, 2, :],
                                        op=Alu.max)
                nc.vector.scalar_tensor_tensor(out=M2[:], in0=M2[:], scalar=2.0,
                                               in1=FLG3[:, 0, :], op0=Alu.mult,
                                               op1=Alu.mult)
                flgs[b] = (FLG, M2)

                OPV = XR[:, T0:T0 + bt, OPA:OPS + 1][:, :, None, :] \
                    .broadcast_to([128, bt, 2, 2])

                # ---------- c build (bf16) ----------
                # cols 0..4 = [a, b, opA, opS, 1]; one fused multiply makes
                # cols 5..9 = cols 0..4 * mA (col 9 = mA since col 4 == 1);
                # cols 10..13 duplicate [a, b, a*mA, b*mA] for the lo-split
                # weight rows.
                nc.vector.tensor_copy(CB4[:, :, 0:2, 0:2], NIBV)
                nc.vector.tensor_copy(CB4[:, :, 0:2, 2:4], OPV)
                nc.vector.tensor_tensor(
                    out=CB4[:, :, 0:2, 5:10], in0=CB4[:, :, 0:2, 0:5],
                    in1=MA[:, :, None, None].broadcast_to([128, bt, 2, 5]),
                    op=Alu.mult)
                nc.vector.tensor_copy(CB4[:, :, 0:2, 10:12], CB4[:, :, 0:2, 0:2])
                nc.vector.tensor_copy(CB4[:, :, 0:2, 12:14], CB4[:, :, 0:2, 5:7])

                # ---------- comp-major via XBAR (one instruction) ----------
                nc.sync.dma_start_transpose(
                    CT[:].rearrange("p (n f) -> p n f", f=128), CB[:])

                rps[b] = rp_pool.tile([128, bt * 4], dt.float32, tag="rp",
                                      name=f"rp{b}")

            def emit_l2(args):
                b, k, RH = args
                for pos in range(2):
                    for j in range(bk):
                        c0 = pos * (bk * 128) + j * 128
                        lc = (k * bk + j) * 4 + pos * 2
                        nc.tensor.matmul(
                            rps[b][:, lc:lc + 2],
                            RH[:, c0:c0 + 128],
                            W2[:],
                            start=True, stop=True)

            def block(b, k):
                CT3 = cts[b % 2][:].rearrange("p (n f) -> p n f", f=128)
                hp = hp_pool.tile([128, bk * 256], dt.float32, tag="hp")
                for pos in range(2):
                    r0 = 32 * pos
                    for j in range(bk):
                        nc.tensor.matmul(
                            hp[:, pos * (bk * 128) + j * 128:
                               pos * (bk * 128) + j * 128 + 128],
                            W16[r0:r0 + 14, :],
                            CT3[r0:r0 + 14, k * bk + j, :],
                            start=True, stop=True,
                            tile_position=(r0, 0))
                RH = rhs_[(b * kpb + k) % 3]
                nc.scalar.activation(RH[:], hp[:], Act.Relu)
                return (b, k, RH)

            def post(b):
                # rp cols per (tile,pos): [res_add - res_sub, res_sub]
                # (difference baked into W2 on the host)
                T0 = b * bt
                FLG, M2 = flgs[b]
                MA = FLG[:].rearrange("p (c n) -> p c n", c=3)[:, 1, :]
                RESS = wpool.tile([128, bt * 4], dt.float32, tag="ress")
                nc.vector.tensor_copy(RESS[:], rps[b][:])
                RESV = RESS[:].rearrange("p (n s w) -> p n s w", s=2, w=2)
                RSEL = wpool.tile([128, bt * 2], dt.float32, tag="rsel")
                RSV = RSEL[:].rearrange("p (n s) -> p n s", s=2)
                # rsel = diff*mA + res_sub
                nc.vector.tensor_tensor(
                    out=RSV, in0=RESV[:, :, :, 0],
                    in1=MA[:, :, None].broadcast_to([128, bt, 2]), op=Alu.mult)
                nc.vector.tensor_tensor(out=RSV, in0=RSV,
                                        in1=RESV[:, :, :, 1], op=Alu.add)
                nc.vector.tensor_scalar(out=RSEL[:], in0=RSEL[:],
                                        scalar1=ROUND_C,
                                        scalar2=ROUND_C - 100.0,
                                        op0=Alu.add, op1=Alu.subtract)
                nc.vector.tensor_scalar(out=RSEL[:], in0=RSEL[:],
                                        scalar1=100.0, scalar2=115.0,
                                        op0=Alu.max, op1=Alu.min)
                RS16 = wpool.tile([128, bt * 2], dt.float16, tag="rs16")
                RS16V = RS16[:].rearrange("p (n s) -> p n s", s=2)
                nc.vector.scalar_tensor_tensor(
                    out=RS16[:].rearrange("p (n s) -> p n s", s=2),
                    in0=M2[:, :, None].broadcast_to([128, bt, 2]),
                    scalar=-50.0, in1=RSV, op0=Alu.mult, op1=Alu.add)
                rsel[b] = RS16
                EQ = wpool.tile([128, bt * 32], dt.float16, tag="eq")
                nc.vector.tensor_tensor(
                    out=EQ[:],
                    in0=IOTA[:].rearrange("p (s k) -> p s k", s=2)[:, None]
                        .broadcast_to([128, bt, 2, 16]),
                    in1=RS16V[:, :, :, None].broadcast_to([128, bt, 2, 16]),
                    op=Alu.is_equal)
                nc.vector.scalar_tensor_tensor(
                    out=XR[:, T0:T0 + bt, OUT_LO:OUT_LO + 32],
                    in0=EQ[:].rearrange("p (n c) -> p n c", c=32),
                    scalar=2.0,
                    in1=XR[:, T0:T0 + bt, OUT_LO:OUT_LO + 32],
                    op0=Alu.mult, op1=Alu.add)
                nc.sync.dma_start(yr[:, T0:T0 + bt, :], XR[:, T0:T0 + bt, :])

            for b in range(nbatch):
                decode(b)
                for k in range(kpb):
                    blk = block(b, k)
                    if pend_l2:
                        emit_l2(pend_l2.pop(0))
                    pend_l2.append(blk)
                if b >= 1:
                    post(b - 1)
            while pend_l2:
                emit_l2(pend_l2.pop(0))
            post(nbatch - 1)

    nc.compile()
    return nc


def make_consts(W_add1, b_add1, W_add2, b_add2, W_sub1, b_sub1, W_sub2, b_sub2):
    f32 = np.float32
    bf16 = ml_dtypes.bfloat16
    rows = [0, 1, 27, 28]     # GE comps: NIB_A, NIB_B, OP_START+25, OP_START+26

    def eff(W1, b1):
        return np.concatenate([np.asarray(W1, f32)[rows, :],
                               np.asarray(b1, f32)[None, :]], axis=0)

    es = eff(W_sub1, b_sub1)
    ea = eff(W_add1, b_add1)
    blk = np.zeros((10, 128), f32)
    blk[0:5] = es
    blk[5:10] = (ea.astype(np.float64) - es.astype(np.float64)).astype(f32)
    bhi = blk.astype(bf16)
    blo = (blk - bhi.astype(f32)).astype(bf16)
    w16 = np.zeros((128, 128), bf16)
    for s in range(4):
        w16[32 * s:32 * s + 10] = bhi
        w16[32 * s + 10:32 * s + 14] = blo[[0, 1, 5, 6]]

    w2a = np.asarray(W_add2, np.float64)[:, GE_RESULT]
    w2s = np.asarray(W_sub2, np.float64)[:, GE_RESULT]
    w2 = np.stack([w2a - w2s, w2s], axis=1).astype(f32).astype(np.float16)

    iota = np.broadcast_to(np.tile(np.arange(16, dtype=np.float16), 2),
                           (128, 32)).copy()
    k16 = np.broadcast_to(((np.arange(64, dtype=f32) % 16) - 16.0)
                          .astype(np.float16), (128, 64)).copy()
    return dict(cW16=w16, cW2=w2, cIOTA=iota, cK16=k16)


_NC_CACHE = {}


def _get_nc(tpc=TPC, bt=BT, bk=BK):
    key = (tpc, bt, bk)
    if key not in _NC_CACHE:
        _NC_CACHE[key] = build_nc(tpc, bt, bk)
    return _NC_CACHE[key]


def kernel(x_bd, W_add1, b_add1, W_add2, b_add2, W_sub1, b_sub1, W_sub2, b_sub2):
    from concourse import bass_utils

    x = np.ascontiguousarray(np.asarray(x_bd, dtype=np.float32)).reshape(TOK, D)
    consts = make_consts(W_add1, b_add1, W_add2, b_add2,
                         W_sub1, b_sub1, W_sub2, b_sub2)
    badd2 = float(np.asarray(b_add2)[GE_RESULT])
    bsub2 = float(np.asarray(b_sub2)[GE_RESULT])
    assert badd2 == 0.0 and bsub2 == 0.0, "nonzero output bias not folded"

    nc = _get_nc()
    in_maps = []
    for c in range(NCORES):
        m = dict(consts)
        m["xc"] = x[c * TPC:(c + 1) * TPC]
        in_maps.append(m)
    res = bass_utils.run_bass_kernel_spmd(nc, in_maps, list(range(NCORES)))
    y = np.concatenate([res.results[c]["yc"] for c in range(NCORES)], axis=0)
    return y.reshape(B, S, D)


if __name__ == "__main__":
    build_nc()
    print("built ok")
